# revision 5
# baseline (speedup 1.0000x reference)
"""Trainium2 Bass kernel for nn_Aggregator1 (GNN message passing).

Strategy (8 NeuronCores, SPMD, no collectives):
  - Destination nodes are split evenly across cores (6250 t-rows, 6250 v-rows,
    2500 a-rows per core). Each core processes exactly the edges that land in
    its destination slice, so no cross-core reduction is needed.
  - Edge gathers use the custom dma_gather instruction (int16 indices,
    4 SWDGE queues round-robin). 50000-row tables are addressed as lo/hi
    halves (25000 rows each) so indices fit int16.
  - Per 128-edge chunk: DVE elementwise multiply, DVE one-hot (is_equal vs
    iota), PE matmul products.T @ onehot accumulating seg.T per 128-dest tile
    in PSUM (transposed output avoids any activation transposes).
  - v-side projections (t_embed @ wt.T, a_embed @ wa_t.T) are computed
    replicated on every core into internal DRAM, then gathered per edge.
  - Dense updates use host-transposed embedding tables as matmul lhsT.
Host-side prep is index manipulation only (no float math on edge data).
"""
import sys
import types

import numpy as np

# ---- shim: provide antenv.axon_hooks (absent in this image) ----
if 'antenv.axon_hooks' not in sys.modules:
    _m = types.ModuleType('antenv.axon_hooks')
    _m._hook = None
    _m.set_axon_ntff_profile_hook = lambda h: setattr(_m, '_hook', h)
    _m.get_axon_ntff_profile_hook = lambda: _m._hook
    sys.modules['antenv.axon_hooks'] = _m
    try:
        from trn_agent_boot.trn_boot import _ntff_profile_via_ctypes
        _m.set_axon_ntff_profile_hook(
            _ntff_profile_via_ctypes('/opt/axon/libaxon_pjrt.so'))
    except Exception:
        pass

import concourse.bass as bass
import concourse.bacc as bacc
import concourse.mybir as mybir
import concourse.tile as tile
import concourse.bass_utils as bass_utils
from concourse.bass_utils import run_bass_kernel_spmd

bass_utils.upload_artifacts = lambda tmpdir: "local://" + str(tmpdir)

P = 128
D = 128
N_T = 50000
N_V = 50000
N_A = 20000
E = 640000
NC = 8
DPC = N_T // NC            # dest nodes per core (t and v sides)
APC = N_A // NC            # a rows per core
TILES = (DPC + P - 1) // P  # dest tiles per core per side (49; last has 106)
HALF = 25000               # lo/hi split for 50000-row gather tables
GROUP = 32                 # chunks per gather group (32*128 = 4096 edges)
F32 = mybir.dt.float32
I16 = mybir.dt.int16


def _side_schedule(ptr, a_list, s_list, seg):
    """Build the per-core chunk schedule for one aggregation side.

    Returns (cpt, streams) where cpt[t][h] is the static chunk count for
    dest-tile t, source-half h, and streams[c][h] are per-core flat arrays
    (a_idx int16, s_idx int16, seg_local float32) laid out chunk-major.
    """
    # per (core, tile, half) edge index arrays
    per = [[[None, None] for _ in range(TILES)] for _ in range(NC)]
    for c in range(NC):
        base_d = c * DPC
        for t in range(TILES):
            d0 = base_d + t * P
            d1 = min(base_d + (t + 1) * P, base_d + DPC)
            e0, e1 = int(ptr[d0]), int(ptr[d1])
            sl = s_list[e0:e1]
            al = a_list[e0:e1]
            dl = seg[e0:e1] - d0  # local dest in [0, d1-d0)
            lo = sl < HALF
            per[c][t][0] = (al[lo], sl[lo], dl[lo])
            per[c][t][1] = (al[~lo], sl[~lo] - HALF, dl[~lo])

    cpt = np.zeros((TILES, 2), dtype=np.int64)
    for t in range(TILES):
        for h in range(2):
            mx = max(len(per[c][t][h][0]) for c in range(NC))
            cpt[t, h] = (mx + P - 1) // P
    # ensure every tile has >= 1 chunk so its PSUM gets start=True zeroing
    for t in range(TILES):
        if cpt[t, 0] + cpt[t, 1] == 0:
            cpt[t, 0] = 1

    nch = [int(cpt[:, h].sum()) for h in range(2)]
    nch_pad = [((n + GROUP - 1) // GROUP) * GROUP for n in nch]

    streams = []
    for c in range(NC):
        halves = []
        for h in range(2):
            ne = nch_pad[h] * P
            a_idx = np.zeros(ne, dtype=np.int16)
            s_idx = np.zeros(ne, dtype=np.int16)
            segl = np.full(ne, -1.0, dtype=np.float32)
            off = 0
            for t in range(TILES):
                al, sl, dl = per[c][t][h]
                n = len(al)
                a_idx[off:off + n] = al
                s_idx[off:off + n] = sl
                segl[off:off + n] = dl
                off += int(cpt[t, h]) * P
            halves.append((a_idx, s_idx, segl))
        streams.append(halves)
    return cpt, nch_pad, streams


def _pack_idx(idx_flat):
    """int16 flat [NE] -> [128, NE//16] tile layout for dma_gather."""
    ne = idx_flat.shape[0]
    arr = idx_flat.reshape(ne // 16, 16).T  # [16, S]
    return np.ascontiguousarray(np.tile(arr, (8, 1)))  # [128, S]


def _pack_seg(seg_flat):
    """f32 flat [NE] -> [128, NCH] (chunk ch, partition p) = seg[ch*128+p]."""
    ne = seg_flat.shape[0]
    return np.ascontiguousarray(seg_flat.reshape(ne // P, P).T)


def _emit_side(nc, tc, pools, side):
    """Emit gathers + multiply + onehot + seg matmuls + dense updates."""
    (cpool, gpool, wpool, ppool) = pools
    cpt = side["cpt"]
    nch_pad = side["nch_pad"]
    qstate = side["qstate"]

    # upfront SBUF loads of idx/seg metadata
    idx_sb = []
    seg_sb = []
    for h in range(2):
        ne = nch_pad[h] * P
        ai = cpool.tile([P, ne // 16], I16, tag=f"{side['name']}_ai{h}")
        si = cpool.tile([P, ne // 16], I16, tag=f"{side['name']}_si{h}")
        sg = cpool.tile([P, nch_pad[h]], F32, tag=f"{side['name']}_sg{h}")
        nc.sync.dma_start(out=ai[:], in_=side["t_aidx"][h][:])
        nc.sync.dma_start(out=si[:], in_=side["t_sidx"][h][:])
        nc.sync.dma_start(out=sg[:], in_=side["t_seg"][h][:])
        idx_sb.append((ai, si))
        seg_sb.append(sg)

    # gathers + multiply per group, per half-stream
    prods = [[], []]
    for h in range(2):
        ngr = nch_pad[h] // GROUP
        ai, si = idx_sb[h]
        for g in range(ngr):
            nidx = GROUP * P
            s0 = g * (nidx // 16)
            s1 = (g + 1) * (nidx // 16)
            ga = gpool.tile([P, GROUP * P], F32, tag=f"ga{h}")
            nc.gpsimd.dma_gather(
                out_ap=ga[:].rearrange("p (k d) -> p k d", d=D),
                in_ap=side["a_table"][:],
                idxs_ap=ai[:, s0:s1],
                num_idxs=nidx, num_idxs_reg=nidx, elem_size=D,
                single_packet=False, queue_num=qstate[0] % 4)
            qstate[0] += 1
            gs = gpool.tile([P, GROUP * P], F32, tag=f"gs{h}")
            nc.gpsimd.dma_gather(
                out_ap=gs[:].rearrange("p (k d) -> p k d", d=D),
                in_ap=side["s_table"][h],
                idxs_ap=si[:, s0:s1],
                num_idxs=nidx, num_idxs_reg=nidx, elem_size=D,
                single_packet=False, queue_num=qstate[0] % 4)
            qstate[0] += 1
            nc.vector.tensor_tensor(out=ga[:], in0=ga[:], in1=gs[:],
                                    op=mybir.AluOpType.mult)
            prods[h].append(ga)

    # per dest-tile: onehot + seg matmuls + dense update
    iota = side["iota"]
    ch_off = [0, 0]
    for t in range(TILES):
        tw = min(P, DPC - t * P)
        psum = ppool.tile([P, P], F32, space="PSUM", tag="pseg")
        nmm = int(cpt[t, 0] + cpt[t, 1])
        mm = 0
        for h in range(2):
            for k in range(int(cpt[t, h])):
                ch = ch_off[h] + k
                g, s = divmod(ch, GROUP)
                oh = wpool.tile([P, P], F32, tag="oh")
                nc.vector.tensor_scalar(
                    oh[:], iota[:], seg_sb[h][:, ch:ch + 1], None,
                    mybir.AluOpType.is_equal)
                nc.tensor.matmul(
                    out=psum[:], lhsT=prods[h][g][:, s * P:(s + 1) * P],
                    rhs=oh[:], start=(mm == 0), stop=(mm == nmm - 1))
                mm += 1
        ch_off[0] += int(cpt[t, 0])
        ch_off[1] += int(cpt[t, 1])

        segT = wpool.tile([P, P], F32, tag="segT")
        nc.scalar.copy(out=segT[:], in_=psum[:])

        # dense update: out[d, j] = seg.T-term + embed-term
        psum_u = ppool.tile([P, P], F32, space="PSUM", tag="pupd")
        nc.tensor.matmul(out=psum_u[:], lhsT=segT[:], rhs=side["w_seg"][:],
                         start=True, stop=False)
        embT = wpool.tile([P, tw], F32, tag="embT")
        nc.sync.dma_start(out=embT[:], in_=side["embT_sl"][:, t * P:t * P + tw])
        nc.tensor.matmul(out=psum_u[:tw, :], lhsT=embT[:], rhs=side["w_emb"][:],
                         start=False, stop=True)
        out_sb = wpool.tile([P, P], F32, tag="outsb")
        nc.scalar.copy(out=out_sb[:tw, :], in_=psum_u[:tw, :])
        nc.sync.dma_start(out=side["out"][t * P:t * P + tw, :],
                          in_=out_sb[:tw, :])


def _build_program(meta):
    """Build the Bass program. meta holds schedules (shared across cores)."""
    nc = bacc.Bacc(num_swdge_queues=4)

    # ---- external inputs ----
    t_a_emb = nc.dram_tensor("a_emb", [N_A, D], F32, kind="ExternalInput")
    t_v_emb = nc.dram_tensor("v_emb", [N_V, D], F32, kind="ExternalInput")
    t_tT = nc.dram_tensor("t_embT", [P, N_T], F32, kind="ExternalInput")
    t_aT = nc.dram_tensor("a_embT", [P, N_A], F32, kind="ExternalInput")
    t_tT_sl = nc.dram_tensor("t_embT_sl", [P, DPC], F32, kind="ExternalInput")
    t_vT_sl = nc.dram_tensor("v_embT_sl", [P, DPC], F32, kind="ExternalInput")
    t_aT_sl = nc.dram_tensor("a_embT_sl", [P, APC], F32, kind="ExternalInput")
    wnames = ["wtT", "watT", "w1aT", "w1bT", "wav", "w2aT", "w2bT", "wa_raw",
              "iota"]
    t_w = {n: nc.dram_tensor(n, [P, P], F32, kind="ExternalInput")
           for n in wnames}

    idx_t = {}
    for sname, sd in (("t", meta["t"]), ("v", meta["v"])):
        for h in range(2):
            ne = sd["nch_pad"][h] * P
            idx_t[(sname, h, "a")] = nc.dram_tensor(
                f"{sname}_aidx{h}", [P, ne // 16], I16, kind="ExternalInput")
            idx_t[(sname, h, "s")] = nc.dram_tensor(
                f"{sname}_sidx{h}", [P, ne // 16], I16, kind="ExternalInput")
            idx_t[(sname, h, "g")] = nc.dram_tensor(
                f"{sname}_seg{h}", [P, sd["nch_pad"][h]], F32,
                kind="ExternalInput")

    # ---- outputs ----
    o_t = nc.dram_tensor("t_upd_part", [DPC, D], F32, kind="ExternalOutput")
    o_v = nc.dram_tensor("v_upd_part", [DPC, D], F32, kind="ExternalOutput")
    o_a = nc.dram_tensor("a_out_part", [APC, D], F32, kind="ExternalOutput")

    # ---- internal DRAM (projections, replicated per core) ----
    d_tproj = nc.dram_tensor("t_proj", [N_T, D], F32)
    d_aproj = nc.dram_tensor("a_proj", [N_A, D], F32)

    with tile.TileContext(nc) as tc:
        with tc.tile_pool(name="consts", bufs=1) as kpool:
            w_sb = {}
            for n in wnames:
                w_sb[n] = kpool.tile([P, P], F32, tag=n, name=n)
                nc.sync.dma_start(out=w_sb[n][:], in_=t_w[n][:])
            # fold w1b_eff.T = (w1[:,128:] @ wa_v).T on device
            with tc.tile_pool(name="fold", bufs=1, space="PSUM") as fpool:
                pf = fpool.tile([P, P], F32, space="PSUM", tag="pf")
                nc.tensor.matmul(out=pf[:], lhsT=w_sb["wav"][:],
                                 rhs=w_sb["w1bT"][:], start=True, stop=True)
                w1beT = kpool.tile([P, P], F32, tag="w1beT")
                nc.scalar.copy(out=w1beT[:], in_=pf[:])

            # ---- phase 1: projections t_proj / a_proj (replicated) ----
            BLK = 2048
            with (
                tc.tile_pool(name="projw", bufs=3) as prw,
                tc.tile_pool(name="projp", bufs=2, space="PSUM") as prp,
            ):
                for (src, dst, n_rows, w_rhs) in (
                        (t_tT, d_tproj, N_T, w_sb["wtT"]),
                        (t_aT, d_aproj, N_A, w_sb["watT"])):
                    nblk = (n_rows + BLK - 1) // BLK
                    for b in range(nblk):
                        r0 = b * BLK
                        bw = min(BLK, n_rows - r0)
                        lhs_big = prw.tile([P, BLK], F32, tag="plhs")
                        nc.sync.dma_start(out=lhs_big[:, :bw],
                                          in_=src[:, r0:r0 + bw])
                        stage = prw.tile([P, BLK], F32, tag="pstg")
                        nsub = (bw + P - 1) // P
                        for qs in range(0, nsub, 4):
                            qe = min(qs + 4, nsub)
                            pp = prp.tile([P, 512], F32, space="PSUM",
                                          tag="ppp")
                            for s in range(qs, qe):
                                sw = min(P, bw - s * P)
                                nc.tensor.matmul(
                                    out=pp[:sw, (s - qs) * P:(s - qs) * P + P],
                                    lhsT=lhs_big[:, s * P:s * P + sw],
                                    rhs=w_rhs[:], start=True, stop=True,
                                    skip_group_check=True)
                            nc.scalar.copy(
                                out=stage[:, qs * P:qs * P + (qe - qs) * P],
                                in_=pp[:, :(qe - qs) * P])
                        if bw % P == 0:
                            nc.sync.dma_start(
                                out=dst[r0:r0 + bw, :].rearrange(
                                    "(s p) d -> p s d", p=P),
                                in_=stage[:, :bw].rearrange(
                                    "p (s d) -> p s d", d=D))
                        else:
                            for s in range(nsub):
                                sw = min(P, bw - s * P)
                                nc.sync.dma_start(
                                    out=dst[r0 + s * P:r0 + s * P + sw, :],
                                    in_=stage[:sw, s * P:(s + 1) * P])

            # ---- phases 2+3: edge aggregation + dense updates ----
            qstate = [0]
            with (
                tc.tile_pool(name="meta", bufs=1) as cpool,
                tc.tile_pool(name="gath", bufs=2) as gpool,
                tc.tile_pool(name="work", bufs=3) as wpool,
                tc.tile_pool(name="psum", bufs=2, space="PSUM") as ppool,
            ):
                side_t = dict(
                    name="t", cpt=meta["t"]["cpt"],
                    nch_pad=meta["t"]["nch_pad"], qstate=qstate,
                    t_aidx=[idx_t[("t", h, "a")] for h in range(2)],
                    t_sidx=[idx_t[("t", h, "s")] for h in range(2)],
                    t_seg=[idx_t[("t", h, "g")] for h in range(2)],
                    a_table=t_a_emb,
                    s_table=[t_v_emb[0:HALF, :], t_v_emb[HALF:, :]],
                    iota=w_sb["iota"], w_seg=w1beT, w_emb=w_sb["w1aT"],
                    embT_sl=t_tT_sl[:], out=o_t)
                _emit_side(nc, tc, (cpool, gpool, wpool, ppool), side_t)

                side_v = dict(
                    name="v", cpt=meta["v"]["cpt"],
                    nch_pad=meta["v"]["nch_pad"], qstate=qstate,
                    t_aidx=[idx_t[("v", h, "a")] for h in range(2)],
                    t_sidx=[idx_t[("v", h, "s")] for h in range(2)],
                    t_seg=[idx_t[("v", h, "g")] for h in range(2)],
                    a_table=d_aproj,
                    s_table=[d_tproj[0:HALF, :], d_tproj[HALF:, :]],
                    iota=w_sb["iota"], w_seg=w_sb["w2bT"],
                    w_emb=w_sb["w2aT"], embT_sl=t_vT_sl[:], out=o_v)
                _emit_side(nc, tc, (cpool, gpool, wpool, ppool), side_v)

                # ---- phase 4: a_out = a_embed @ wa (sharded rows) ----
                ntile_a = (APC + P - 1) // P
                for i in range(ntile_a):
                    r0 = i * P
                    tw = min(P, APC - r0)
                    pa = ppool.tile([P, P], F32, space="PSUM", tag="pupd")
                    lhs = wpool.tile([P, tw], F32, tag="embT")
                    nc.sync.dma_start(out=lhs[:], in_=t_aT_sl[:, r0:r0 + tw])
                    nc.tensor.matmul(out=pa[:tw, :], lhsT=lhs[:],
                                     rhs=w_sb["wa_raw"][:], start=True,
                                     stop=True)
                    oa = wpool.tile([P, P], F32, tag="outsb")
                    nc.scalar.copy(out=oa[:tw, :], in_=pa[:tw, :])
                    nc.sync.dma_start(out=o_a[r0:r0 + tw, :], in_=oa[:tw, :])

    nc.compile()
    return nc


def _host_prep(inputs):
    """Index-only preprocessing; returns (meta, in_maps)."""
    ptr_t = np.asarray(inputs["ptr_t"])
    ptr_v = np.asarray(inputs["ptr_v"])
    a_l_t = np.asarray(inputs["a_list_t"])
    v_l_t = np.asarray(inputs["v_list_t"])
    a_l_v = np.asarray(inputs["a_list_v"])
    t_l_v = np.asarray(inputs["t_list_v"])
    ar = np.arange(E)
    seg_t = np.searchsorted(ptr_t, ar, side='right') - 1
    seg_v = np.searchsorted(ptr_v, ar, side='right') - 1

    meta = {}
    packed = {}
    for sname, (ptr, al, sl, seg) in (
            ("t", (ptr_t, a_l_t, v_l_t, seg_t)),
            ("v", (ptr_v, a_l_v, t_l_v, seg_v))):
        cpt, nch_pad, streams = _side_schedule(ptr, al, sl, seg)
        meta[sname] = {"cpt": cpt, "nch_pad": nch_pad}
        packed[sname] = streams

    t_embed = np.asarray(inputs["t_embed"], dtype=np.float32)
    v_embed = np.asarray(inputs["v_embed"], dtype=np.float32)
    a_embed = np.asarray(inputs["a_embed"], dtype=np.float32)
    wt = np.asarray(inputs["wt"], dtype=np.float32)
    wa_t = np.asarray(inputs["wa_t"], dtype=np.float32)
    wa_v = np.asarray(inputs["wa_v"], dtype=np.float32)
    w1 = np.asarray(inputs["w1"], dtype=np.float32)
    w2 = np.asarray(inputs["w2"], dtype=np.float32)
    wa = np.asarray(inputs["wa"], dtype=np.float32)

    tT = np.ascontiguousarray(t_embed.T)
    vT = np.ascontiguousarray(v_embed.T)
    aT = np.ascontiguousarray(a_embed.T)
    iota = np.ascontiguousarray(
        np.tile(np.arange(P, dtype=np.float32)[None, :], (P, 1)))

    common = {
        "a_emb": a_embed, "v_emb": v_embed,
        "t_embT": tT, "a_embT": aT,
        "wtT": np.ascontiguousarray(wt.T),
        "watT": np.ascontiguousarray(wa_t.T),
        "w1aT": np.ascontiguousarray(w1[:, :D].T),
        "w1bT": np.ascontiguousarray(w1[:, D:].T),
        "wav": wa_v,
        "w2aT": np.ascontiguousarray(w2[:, :D].T),
        "w2bT": np.ascontiguousarray(w2[:, D:].T),
        "wa_raw": wa,
        "iota": iota,
    }

    in_maps = []
    for c in range(NC):
        m = dict(common)
        m["t_embT_sl"] = np.ascontiguousarray(tT[:, c * DPC:(c + 1) * DPC])
        m["v_embT_sl"] = np.ascontiguousarray(vT[:, c * DPC:(c + 1) * DPC])
        m["a_embT_sl"] = np.ascontiguousarray(aT[:, c * APC:(c + 1) * APC])
        for sname in ("t", "v"):
            for h in range(2):
                a_idx, s_idx, segl = packed[sname][c][h]
                m[f"{sname}_aidx{h}"] = _pack_idx(a_idx)
                m[f"{sname}_sidx{h}"] = _pack_idx(s_idx)
                m[f"{sname}_seg{h}"] = _pack_seg(segl)
        in_maps.append(m)
    return meta, in_maps


_CACHE = {}


def _get_compiled(inputs):
    key = (inputs["ptr_t"].tobytes()[:4096], inputs["ptr_v"].tobytes()[:4096],
           inputs["a_list_t"].tobytes()[:4096])
    import hashlib
    key = hashlib.sha1(b"".join(key)).hexdigest()
    if key not in _CACHE:
        meta, in_maps = _host_prep(inputs)
        nc = _build_program(meta)
        _CACHE[key] = (nc, meta)
    else:
        nc, meta = _CACHE[key]
        _, in_maps = _host_prep(inputs)
    return _CACHE[key][0], in_maps


def run(inputs, trace=False):
    nc, in_maps = _get_compiled(inputs)
    res = run_bass_kernel_spmd(nc, in_maps, list(range(NC)), trace=trace)
    t_upd = np.concatenate([res.results[c]["t_upd_part"] for c in range(NC)])
    v_upd = np.concatenate([res.results[c]["v_upd_part"] for c in range(NC)])
    a_out = np.concatenate([res.results[c]["a_out_part"] for c in range(NC)])
    return (t_upd, v_upd, a_out), res


def kernel(**inputs):
    out, _ = run(inputs, trace=False)
    return out


# revision 7
# speedup vs baseline: 1.2336x; 1.2336x over previous
"""Trainium2 Bass kernel for nn_Aggregator1 (GNN message passing).

Strategy (8 NeuronCores, SPMD, no collectives):
  - Destination nodes are split evenly across cores (6250 t-rows, 6250 v-rows,
    2500 a-rows per core). Each core processes exactly the edges that land in
    its destination slice, so no cross-core reduction is needed.
  - Edge gathers use the custom dma_gather instruction (int16 indices,
    4 SWDGE queues round-robin). 50000-row tables are addressed as lo/hi
    halves (25000 rows each) so indices fit int16.
  - Per 128-edge chunk: DVE elementwise multiply, DVE one-hot (is_equal vs
    iota), PE matmul products.T @ onehot accumulating seg.T per 128-dest tile
    in PSUM (transposed output avoids any activation transposes).
  - v-side projections (t_embed @ wt.T, a_embed @ wa_t.T) are computed
    replicated on every core into internal DRAM, then gathered per edge.
  - Dense updates use host-transposed embedding tables as matmul lhsT.
Host-side prep is index manipulation only (no float math on edge data).
"""
import sys
import types

import numpy as np

# ---- shim: provide antenv.axon_hooks (absent in this image) ----
if 'antenv.axon_hooks' not in sys.modules:
    _m = types.ModuleType('antenv.axon_hooks')
    _m._hook = None
    _m.set_axon_ntff_profile_hook = lambda h: setattr(_m, '_hook', h)
    _m.get_axon_ntff_profile_hook = lambda: _m._hook
    sys.modules['antenv.axon_hooks'] = _m
    try:
        from trn_agent_boot.trn_boot import _ntff_profile_via_ctypes
        _m.set_axon_ntff_profile_hook(
            _ntff_profile_via_ctypes('/opt/axon/libaxon_pjrt.so'))
    except Exception:
        pass

import concourse.bass as bass
import concourse.bacc as bacc
import concourse.mybir as mybir
import concourse.tile as tile
import concourse.bass_utils as bass_utils
from concourse.bass_utils import run_bass_kernel_spmd

bass_utils.upload_artifacts = lambda tmpdir: "local://" + str(tmpdir)

P = 128
D = 128
N_T = 50000
N_V = 50000
N_A = 20000
E = 640000
NC = 8
DPC = N_T // NC            # dest nodes per core (t and v sides)
APC = N_A // NC            # a rows per core
TILES = (DPC + P - 1) // P  # dest tiles per core per side (49; last has 106)
HALF = 25000               # lo/hi split for 50000-row gather tables
GROUP = 32                 # chunks per gather group (32*128 = 4096 edges)
F32 = mybir.dt.float32
I16 = mybir.dt.int16


def _side_schedule(ptr, a_list, s_list, seg):
    """Build the per-core chunk schedule for one aggregation side.

    Returns (cpt, streams) where cpt[t][h] is the static chunk count for
    dest-tile t, source-half h, and streams[c][h] are per-core flat arrays
    (a_idx int16, s_idx int16, seg_local float32) laid out chunk-major.
    """
    # per (core, tile, half) edge index arrays
    per = [[[None, None] for _ in range(TILES)] for _ in range(NC)]
    for c in range(NC):
        base_d = c * DPC
        for t in range(TILES):
            d0 = base_d + t * P
            d1 = min(base_d + (t + 1) * P, base_d + DPC)
            e0, e1 = int(ptr[d0]), int(ptr[d1])
            sl = s_list[e0:e1]
            al = a_list[e0:e1]
            dl = seg[e0:e1] - d0  # local dest in [0, d1-d0)
            lo = sl < HALF
            per[c][t][0] = (al[lo], sl[lo], dl[lo])
            per[c][t][1] = (al[~lo], sl[~lo] - HALF, dl[~lo])

    cpt = np.zeros((TILES, 2), dtype=np.int64)
    for t in range(TILES):
        for h in range(2):
            mx = max(len(per[c][t][h][0]) for c in range(NC))
            cpt[t, h] = (mx + P - 1) // P
    # ensure every tile has >= 1 chunk so its PSUM gets start=True zeroing
    for t in range(TILES):
        if cpt[t, 0] + cpt[t, 1] == 0:
            cpt[t, 0] = 1

    nch = [int(cpt[:, h].sum()) for h in range(2)]
    nch_pad = [((n + GROUP - 1) // GROUP) * GROUP for n in nch]

    streams = []
    for c in range(NC):
        halves = []
        for h in range(2):
            ne = nch_pad[h] * P
            a_idx = np.zeros(ne, dtype=np.int16)
            s_idx = np.zeros(ne, dtype=np.int16)
            segl = np.full(ne, -1.0, dtype=np.float32)
            off = 0
            for t in range(TILES):
                al, sl, dl = per[c][t][h]
                n = len(al)
                a_idx[off:off + n] = al
                s_idx[off:off + n] = sl
                segl[off:off + n] = dl
                off += int(cpt[t, h]) * P
            halves.append((a_idx, s_idx, segl))
        streams.append(halves)
    return cpt, nch_pad, streams


def _pack_idx(idx_flat):
    """int16 flat [NE] -> [128, NE//16] tile layout for dma_gather."""
    ne = idx_flat.shape[0]
    arr = idx_flat.reshape(ne // 16, 16).T  # [16, S]
    return np.ascontiguousarray(np.tile(arr, (8, 1)))  # [128, S]


def _pack_seg(seg_flat):
    """f32 flat [NE] -> [128, NCH] (chunk ch, partition p) = seg[ch*128+p]."""
    ne = seg_flat.shape[0]
    return np.ascontiguousarray(seg_flat.reshape(ne // P, P).T)


OHB = 16  # chunks per batched one-hot op


def _emit_side(nc, tc, pools, side):
    """Emit gathers + multiply + onehot + seg matmuls + dense updates."""
    (gpool, wpool, ppool) = pools
    cpt = side["cpt"]
    nch_pad = side["nch_pad"]
    qstate = side["qstate"]
    mctx = tc.tile_pool(name=f"meta_{side['name']}", bufs=1)
    mpool = mctx.__enter__()

    # upfront SBUF loads of idx/seg metadata
    idx_sb = []
    seg_sb = []
    for h in range(2):
        ne = nch_pad[h] * P
        ai = mpool.tile([P, ne // 16], I16, tag=f"ai{h}", name=f"ai{h}")
        si = mpool.tile([P, ne // 16], I16, tag=f"si{h}", name=f"si{h}")
        sg = mpool.tile([P, nch_pad[h]], F32, tag=f"sg{h}", name=f"sg{h}")
        nc.sync.dma_start(out=ai[:], in_=side["t_aidx"][h][:])
        nc.sync.dma_start(out=si[:], in_=side["t_sidx"][h][:])
        nc.sync.dma_start(out=sg[:], in_=side["t_seg"][h][:])
        idx_sb.append((ai, si))
        seg_sb.append(sg)

    # gathers + multiply per group, per half-stream
    prods = [[], []]
    for h in range(2):
        ngr = nch_pad[h] // GROUP
        ai, si = idx_sb[h]
        for g in range(ngr):
            nidx = GROUP * P
            s0 = g * (nidx // 16)
            s1 = (g + 1) * (nidx // 16)
            ga = gpool.tile([P, GROUP * P], F32, tag=f"ga{h}")
            nc.gpsimd.dma_gather(
                out_ap=ga[:].rearrange("p (k d) -> p k d", d=D),
                in_ap=side["a_table"][:],
                idxs_ap=ai[:, s0:s1],
                num_idxs=nidx, num_idxs_reg=nidx, elem_size=D,
                single_packet=False, queue_num=qstate[0] % 4)
            qstate[0] += 1
            gs = gpool.tile([P, GROUP * P], F32, tag=f"gs{h}")
            nc.gpsimd.dma_gather(
                out_ap=gs[:].rearrange("p (k d) -> p k d", d=D),
                in_ap=side["s_table"][h],
                idxs_ap=si[:, s0:s1],
                num_idxs=nidx, num_idxs_reg=nidx, elem_size=D,
                single_packet=False, queue_num=qstate[0] % 4)
            qstate[0] += 1
            nc.vector.tensor_tensor(out=ga[:], in0=ga[:], in1=gs[:],
                                    op=mybir.AluOpType.mult)
            prods[h].append(ga)

    # batched one-hots: oh_batch[h][b] covers chunks [b*OHB, (b+1)*OHB)
    iota = side["iota"]
    oh_batches = [{}, {}]

    def get_oh(h, b):
        if b not in oh_batches[h]:
            k = min(OHB, nch_pad[h] - b * OHB)
            ohb = wpool.tile([P, OHB * P], F32, tag="ohb", bufs=4)
            seg_b = seg_sb[h][:, b * OHB:b * OHB + k] \
                .rearrange("p (k one) -> p k one", one=1) \
                .to_broadcast([P, k, P])
            iota_b = iota[:].rearrange("p (one d) -> p one d", one=1) \
                .to_broadcast([P, k, P])
            nc.vector.tensor_tensor(
                out=ohb[:, :k * P].rearrange("p (k d) -> p k d", d=P),
                in0=iota_b, in1=seg_b, op=mybir.AluOpType.is_equal)
            oh_batches[h][b] = ohb
        return oh_batches[h][b]

    # per dest-tile: seg matmuls + dense update
    ch_off = [0, 0]
    for t in range(TILES):
        tw = min(P, DPC - t * P)
        psum = ppool.tile([P, P], F32, space="PSUM", tag="pseg")
        nmm = int(cpt[t, 0] + cpt[t, 1])
        mm = 0
        for h in range(2):
            for k in range(int(cpt[t, h])):
                ch = ch_off[h] + k
                g, s = divmod(ch, GROUP)
                b, r = divmod(ch, OHB)
                ohb = get_oh(h, b)
                nc.tensor.matmul(
                    out=psum[:], lhsT=prods[h][g][:, s * P:(s + 1) * P],
                    rhs=ohb[:, r * P:(r + 1) * P],
                    start=(mm == 0), stop=(mm == nmm - 1))
                mm += 1
        ch_off[0] += int(cpt[t, 0])
        ch_off[1] += int(cpt[t, 1])

        segT = wpool.tile([P, P], F32, tag="segT")
        nc.scalar.copy(out=segT[:], in_=psum[:])

        # dense update: out[d, j] = seg.T-term + embed-term
        psum_u = ppool.tile([P, P], F32, space="PSUM", tag="pupd")
        nc.tensor.matmul(out=psum_u[:], lhsT=segT[:], rhs=side["w_seg"][:],
                         start=True, stop=False)
        embT = wpool.tile([P, tw], F32, tag="embT")
        nc.sync.dma_start(out=embT[:], in_=side["embT_sl"][:, t * P:t * P + tw])
        nc.tensor.matmul(out=psum_u[:tw, :], lhsT=embT[:], rhs=side["w_emb"][:],
                         start=False, stop=True)
        out_sb = wpool.tile([P, P], F32, tag="outsb")
        nc.scalar.copy(out=out_sb[:tw, :], in_=psum_u[:tw, :])
        nc.sync.dma_start(out=side["out"][t * P:t * P + tw, :],
                          in_=out_sb[:tw, :])
    mctx.__exit__(None, None, None)


def _build_program(meta):
    """Build the Bass program. meta holds schedules (shared across cores)."""
    nc = bacc.Bacc(num_swdge_queues=4)

    # ---- external inputs ----
    t_a_emb = nc.dram_tensor("a_emb", [N_A, D], F32, kind="ExternalInput")
    t_v_emb = nc.dram_tensor("v_emb", [N_V, D], F32, kind="ExternalInput")
    t_tT = nc.dram_tensor("t_embT", [P, N_T], F32, kind="ExternalInput")
    t_aT = nc.dram_tensor("a_embT", [P, N_A], F32, kind="ExternalInput")
    t_tT_sl = nc.dram_tensor("t_embT_sl", [P, DPC], F32, kind="ExternalInput")
    t_vT_sl = nc.dram_tensor("v_embT_sl", [P, DPC], F32, kind="ExternalInput")
    t_aT_sl = nc.dram_tensor("a_embT_sl", [P, APC], F32, kind="ExternalInput")
    wnames = ["wtT", "watT", "w1aT", "w1bT", "wav", "w2aT", "w2bT", "wa_raw",
              "iota"]
    t_w = {n: nc.dram_tensor(n, [P, P], F32, kind="ExternalInput")
           for n in wnames}

    idx_t = {}
    for sname, sd in (("t", meta["t"]), ("v", meta["v"])):
        for h in range(2):
            ne = sd["nch_pad"][h] * P
            idx_t[(sname, h, "a")] = nc.dram_tensor(
                f"{sname}_aidx{h}", [P, ne // 16], I16, kind="ExternalInput")
            idx_t[(sname, h, "s")] = nc.dram_tensor(
                f"{sname}_sidx{h}", [P, ne // 16], I16, kind="ExternalInput")
            idx_t[(sname, h, "g")] = nc.dram_tensor(
                f"{sname}_seg{h}", [P, sd["nch_pad"][h]], F32,
                kind="ExternalInput")

    # ---- outputs ----
    o_t = nc.dram_tensor("t_upd_part", [DPC, D], F32, kind="ExternalOutput")
    o_v = nc.dram_tensor("v_upd_part", [DPC, D], F32, kind="ExternalOutput")
    o_a = nc.dram_tensor("a_out_part", [APC, D], F32, kind="ExternalOutput")

    # ---- internal DRAM (projections, replicated per core) ----
    d_tproj = nc.dram_tensor("t_proj", [N_T, D], F32)
    d_aproj = nc.dram_tensor("a_proj", [N_A, D], F32)

    with tile.TileContext(nc) as tc:
        with tc.tile_pool(name="consts", bufs=1) as kpool:
            w_sb = {}
            for n in wnames:
                w_sb[n] = kpool.tile([P, P], F32, tag=n, name=n)
                nc.sync.dma_start(out=w_sb[n][:], in_=t_w[n][:])
            # fold w1b_eff.T = (w1[:,128:] @ wa_v).T on device
            with tc.tile_pool(name="fold", bufs=1, space="PSUM") as fpool:
                pf = fpool.tile([P, P], F32, space="PSUM", tag="pf")
                nc.tensor.matmul(out=pf[:], lhsT=w_sb["wav"][:],
                                 rhs=w_sb["w1bT"][:], start=True, stop=True)
                w1beT = kpool.tile([P, P], F32, tag="w1beT")
                nc.scalar.copy(out=w1beT[:], in_=pf[:])

            # ---- phase 1: projections t_proj / a_proj (replicated) ----
            BLK = 2048
            with (
                tc.tile_pool(name="projw", bufs=3) as prw,
                tc.tile_pool(name="projp", bufs=2, space="PSUM") as prp,
            ):
                for (src, dst, n_rows, w_rhs) in (
                        (t_tT, d_tproj, N_T, w_sb["wtT"]),
                        (t_aT, d_aproj, N_A, w_sb["watT"])):
                    nblk = (n_rows + BLK - 1) // BLK
                    for b in range(nblk):
                        r0 = b * BLK
                        bw = min(BLK, n_rows - r0)
                        lhs_big = prw.tile([P, BLK], F32, tag="plhs")
                        nc.sync.dma_start(out=lhs_big[:, :bw],
                                          in_=src[:, r0:r0 + bw])
                        stage = prw.tile([P, BLK], F32, tag="pstg")
                        nsub = (bw + P - 1) // P
                        for qs in range(0, nsub, 4):
                            qe = min(qs + 4, nsub)
                            pp = prp.tile([P, 512], F32, space="PSUM",
                                          tag="ppp")
                            for s in range(qs, qe):
                                sw = min(P, bw - s * P)
                                nc.tensor.matmul(
                                    out=pp[:sw, (s - qs) * P:(s - qs) * P + P],
                                    lhsT=lhs_big[:, s * P:s * P + sw],
                                    rhs=w_rhs[:], start=True, stop=True,
                                    skip_group_check=True)
                            nc.scalar.copy(
                                out=stage[:, qs * P:qs * P + (qe - qs) * P],
                                in_=pp[:, :(qe - qs) * P])
                        if bw % P == 0:
                            nc.sync.dma_start(
                                out=dst[r0:r0 + bw, :].rearrange(
                                    "(s p) d -> p s d", p=P),
                                in_=stage[:, :bw].rearrange(
                                    "p (s d) -> p s d", d=D))
                        else:
                            for s in range(nsub):
                                sw = min(P, bw - s * P)
                                nc.sync.dma_start(
                                    out=dst[r0 + s * P:r0 + s * P + sw, :],
                                    in_=stage[:sw, s * P:(s + 1) * P])

            # ---- phases 2+3: edge aggregation + dense updates ----
            qstate = [0]
            with (
                tc.tile_pool(name="gath", bufs=2) as gpool,
                tc.tile_pool(name="work", bufs=3) as wpool,
                tc.tile_pool(name="psum", bufs=2, space="PSUM") as ppool,
            ):
                side_t = dict(
                    name="t", cpt=meta["t"]["cpt"],
                    nch_pad=meta["t"]["nch_pad"], qstate=qstate,
                    t_aidx=[idx_t[("t", h, "a")] for h in range(2)],
                    t_sidx=[idx_t[("t", h, "s")] for h in range(2)],
                    t_seg=[idx_t[("t", h, "g")] for h in range(2)],
                    a_table=t_a_emb,
                    s_table=[t_v_emb[0:HALF, :], t_v_emb[HALF:, :]],
                    iota=w_sb["iota"], w_seg=w1beT, w_emb=w_sb["w1aT"],
                    embT_sl=t_tT_sl[:], out=o_t)
                with nc.named_scope("tside"):
                    _emit_side(nc, tc, (gpool, wpool, ppool), side_t)

                side_v = dict(
                    name="v", cpt=meta["v"]["cpt"],
                    nch_pad=meta["v"]["nch_pad"], qstate=qstate,
                    t_aidx=[idx_t[("v", h, "a")] for h in range(2)],
                    t_sidx=[idx_t[("v", h, "s")] for h in range(2)],
                    t_seg=[idx_t[("v", h, "g")] for h in range(2)],
                    a_table=d_aproj,
                    s_table=[d_tproj[0:HALF, :], d_tproj[HALF:, :]],
                    iota=w_sb["iota"], w_seg=w_sb["w2bT"],
                    w_emb=w_sb["w2aT"], embT_sl=t_vT_sl[:], out=o_v)
                with nc.named_scope("vside"):
                    _emit_side(nc, tc, (gpool, wpool, ppool), side_v)

                # ---- phase 4: a_out = a_embed @ wa (sharded rows) ----
                ntile_a = (APC + P - 1) // P
                for i in range(ntile_a):
                    r0 = i * P
                    tw = min(P, APC - r0)
                    pa = ppool.tile([P, P], F32, space="PSUM", tag="pupd")
                    lhs = wpool.tile([P, tw], F32, tag="embT")
                    nc.sync.dma_start(out=lhs[:], in_=t_aT_sl[:, r0:r0 + tw])
                    nc.tensor.matmul(out=pa[:tw, :], lhsT=lhs[:],
                                     rhs=w_sb["wa_raw"][:], start=True,
                                     stop=True)
                    oa = wpool.tile([P, P], F32, tag="outsb")
                    nc.scalar.copy(out=oa[:tw, :], in_=pa[:tw, :])
                    nc.sync.dma_start(out=o_a[r0:r0 + tw, :], in_=oa[:tw, :])

    nc.compile()
    return nc


def _host_prep(inputs):
    """Index-only preprocessing; returns (meta, in_maps)."""
    ptr_t = np.asarray(inputs["ptr_t"])
    ptr_v = np.asarray(inputs["ptr_v"])
    a_l_t = np.asarray(inputs["a_list_t"])
    v_l_t = np.asarray(inputs["v_list_t"])
    a_l_v = np.asarray(inputs["a_list_v"])
    t_l_v = np.asarray(inputs["t_list_v"])
    ar = np.arange(E)
    seg_t = np.searchsorted(ptr_t, ar, side='right') - 1
    seg_v = np.searchsorted(ptr_v, ar, side='right') - 1

    meta = {}
    packed = {}
    for sname, (ptr, al, sl, seg) in (
            ("t", (ptr_t, a_l_t, v_l_t, seg_t)),
            ("v", (ptr_v, a_l_v, t_l_v, seg_v))):
        cpt, nch_pad, streams = _side_schedule(ptr, al, sl, seg)
        meta[sname] = {"cpt": cpt, "nch_pad": nch_pad}
        packed[sname] = streams

    t_embed = np.asarray(inputs["t_embed"], dtype=np.float32)
    v_embed = np.asarray(inputs["v_embed"], dtype=np.float32)
    a_embed = np.asarray(inputs["a_embed"], dtype=np.float32)
    wt = np.asarray(inputs["wt"], dtype=np.float32)
    wa_t = np.asarray(inputs["wa_t"], dtype=np.float32)
    wa_v = np.asarray(inputs["wa_v"], dtype=np.float32)
    w1 = np.asarray(inputs["w1"], dtype=np.float32)
    w2 = np.asarray(inputs["w2"], dtype=np.float32)
    wa = np.asarray(inputs["wa"], dtype=np.float32)

    tT = np.ascontiguousarray(t_embed.T)
    vT = np.ascontiguousarray(v_embed.T)
    aT = np.ascontiguousarray(a_embed.T)
    iota = np.ascontiguousarray(
        np.tile(np.arange(P, dtype=np.float32)[None, :], (P, 1)))

    common = {
        "a_emb": a_embed, "v_emb": v_embed,
        "t_embT": tT, "a_embT": aT,
        "wtT": np.ascontiguousarray(wt.T),
        "watT": np.ascontiguousarray(wa_t.T),
        "w1aT": np.ascontiguousarray(w1[:, :D].T),
        "w1bT": np.ascontiguousarray(w1[:, D:].T),
        "wav": wa_v,
        "w2aT": np.ascontiguousarray(w2[:, :D].T),
        "w2bT": np.ascontiguousarray(w2[:, D:].T),
        "wa_raw": wa,
        "iota": iota,
    }

    in_maps = []
    for c in range(NC):
        m = dict(common)
        m["t_embT_sl"] = np.ascontiguousarray(tT[:, c * DPC:(c + 1) * DPC])
        m["v_embT_sl"] = np.ascontiguousarray(vT[:, c * DPC:(c + 1) * DPC])
        m["a_embT_sl"] = np.ascontiguousarray(aT[:, c * APC:(c + 1) * APC])
        for sname in ("t", "v"):
            for h in range(2):
                a_idx, s_idx, segl = packed[sname][c][h]
                m[f"{sname}_aidx{h}"] = _pack_idx(a_idx)
                m[f"{sname}_sidx{h}"] = _pack_idx(s_idx)
                m[f"{sname}_seg{h}"] = _pack_seg(segl)
        in_maps.append(m)
    return meta, in_maps


_CACHE = {}


def _get_compiled(inputs):
    key = (inputs["ptr_t"].tobytes()[:4096], inputs["ptr_v"].tobytes()[:4096],
           inputs["a_list_t"].tobytes()[:4096])
    import hashlib
    key = hashlib.sha1(b"".join(key)).hexdigest()
    if key not in _CACHE:
        meta, in_maps = _host_prep(inputs)
        nc = _build_program(meta)
        _CACHE[key] = (nc, meta)
    else:
        nc, meta = _CACHE[key]
        _, in_maps = _host_prep(inputs)
    return _CACHE[key][0], in_maps


def run(inputs, trace=False):
    nc, in_maps = _get_compiled(inputs)
    res = run_bass_kernel_spmd(nc, in_maps, list(range(NC)), trace=trace)
    t_upd = np.concatenate([res.results[c]["t_upd_part"] for c in range(NC)])
    v_upd = np.concatenate([res.results[c]["v_upd_part"] for c in range(NC)])
    a_out = np.concatenate([res.results[c]["a_out_part"] for c in range(NC)])
    return (t_upd, v_upd, a_out), res


def kernel(**inputs):
    out, _ = run(inputs, trace=False)
    return out


# revision 9
# speedup vs baseline: 1.5010x; 1.2167x over previous
"""Trainium2 Bass kernel for nn_Aggregator1 (GNN message passing).

Strategy (8 NeuronCores, SPMD, no collectives):
  - Destination nodes are split evenly across cores (6250 t-rows, 6250 v-rows,
    2500 a-rows per core). Each core processes exactly the edges that land in
    its destination slice, so no cross-core reduction is needed.
  - Edge gathers use the custom dma_gather instruction (int16 indices,
    4 SWDGE queues round-robin). 50000-row tables are addressed as lo/hi
    halves (25000 rows each) so indices fit int16.
  - Per 128-edge chunk: DVE elementwise multiply, DVE one-hot (is_equal vs
    iota), PE matmul products.T @ onehot accumulating seg.T per 128-dest tile
    in PSUM (transposed output avoids any activation transposes).
  - v-side projections (t_embed @ wt.T, a_embed @ wa_t.T) are computed
    replicated on every core into internal DRAM, then gathered per edge.
  - Dense updates use host-transposed embedding tables as matmul lhsT.
Host-side prep is index manipulation only (no float math on edge data).
"""
import sys
import types

import numpy as np

# ---- shim: provide antenv.axon_hooks (absent in this image) ----
if 'antenv.axon_hooks' not in sys.modules:
    _m = types.ModuleType('antenv.axon_hooks')
    _m._hook = None
    _m.set_axon_ntff_profile_hook = lambda h: setattr(_m, '_hook', h)
    _m.get_axon_ntff_profile_hook = lambda: _m._hook
    sys.modules['antenv.axon_hooks'] = _m
    try:
        from trn_agent_boot.trn_boot import _ntff_profile_via_ctypes
        _m.set_axon_ntff_profile_hook(
            _ntff_profile_via_ctypes('/opt/axon/libaxon_pjrt.so'))
    except Exception:
        pass

import concourse.bass as bass
import concourse.bacc as bacc
import concourse.mybir as mybir
import concourse.tile as tile
import concourse.bass_utils as bass_utils
from concourse.bass_utils import run_bass_kernel_spmd

bass_utils.upload_artifacts = lambda tmpdir: "local://" + str(tmpdir)

P = 128
D = 128
N_T = 50000
N_V = 50000
N_A = 20000
E = 640000
NC = 8
DPC = N_T // NC            # dest nodes per core (t and v sides)
APC = N_A // NC            # a rows per core
TILES = (DPC + P - 1) // P  # dest tiles per core per side (49; last has 106)
HALF = 25000               # lo/hi split for 50000-row gather tables
GROUP = 16                 # chunks per gather group (16*128 = 2048 edges)
F32 = mybir.dt.float32
I16 = mybir.dt.int16


def _side_schedule(ptr, a_list, s_list, seg):
    """Build the per-core chunk schedule for one aggregation side.

    Returns (cpt, streams) where cpt[t][h] is the static chunk count for
    dest-tile t, source-half h, and streams[c][h] are per-core flat arrays
    (a_idx int16, s_idx int16, seg_local float32) laid out chunk-major.
    """
    # per (core, tile, half) edge index arrays
    per = [[[None, None] for _ in range(TILES)] for _ in range(NC)]
    for c in range(NC):
        base_d = c * DPC
        for t in range(TILES):
            d0 = base_d + t * P
            d1 = min(base_d + (t + 1) * P, base_d + DPC)
            e0, e1 = int(ptr[d0]), int(ptr[d1])
            sl = s_list[e0:e1]
            al = a_list[e0:e1]
            dl = seg[e0:e1] - d0  # local dest in [0, d1-d0)
            lo = sl < HALF
            per[c][t][0] = (al[lo], sl[lo], dl[lo])
            per[c][t][1] = (al[~lo], sl[~lo] - HALF, dl[~lo])

    cpt = np.zeros((TILES, 2), dtype=np.int64)
    for t in range(TILES):
        for h in range(2):
            mx = max(len(per[c][t][h][0]) for c in range(NC))
            cpt[t, h] = (mx + P - 1) // P
    # ensure every tile has >= 1 chunk so its PSUM gets start=True zeroing
    for t in range(TILES):
        if cpt[t, 0] + cpt[t, 1] == 0:
            cpt[t, 0] = 1

    nch = [int(cpt[:, h].sum()) for h in range(2)]
    nch_pad = [((n + GROUP - 1) // GROUP) * GROUP for n in nch]

    streams = []
    for c in range(NC):
        halves = []
        for h in range(2):
            ne = nch_pad[h] * P
            a_idx = np.zeros(ne, dtype=np.int16)
            s_idx = np.zeros(ne, dtype=np.int16)
            segl = np.full(ne, -1.0, dtype=np.float32)
            off = 0
            for t in range(TILES):
                al, sl, dl = per[c][t][h]
                n = len(al)
                a_idx[off:off + n] = al
                s_idx[off:off + n] = sl
                segl[off:off + n] = dl
                off += int(cpt[t, h]) * P
            halves.append((a_idx, s_idx, segl))
        streams.append(halves)
    return cpt, nch_pad, streams


def _pack_idx(idx_flat):
    """int16 flat [NE] -> [128, NE//16] tile layout for dma_gather."""
    ne = idx_flat.shape[0]
    arr = idx_flat.reshape(ne // 16, 16).T  # [16, S]
    return np.ascontiguousarray(np.tile(arr, (8, 1)))  # [128, S]


def _pack_seg(seg_flat):
    """f32 flat [NE] -> [128, NCH] (chunk ch, partition p) = seg[ch*128+p]."""
    ne = seg_flat.shape[0]
    return np.ascontiguousarray(seg_flat.reshape(ne // P, P).T)


OHB = 16  # chunks per batched one-hot op


def _emit_side(nc, tc, pools, side):
    """Emit gathers + multiply + onehot + seg matmuls + dense updates."""
    (gpool, wpool, ppool) = pools
    cpt = side["cpt"]
    nch_pad = side["nch_pad"]
    qstate = side["qstate"]
    mctx = tc.tile_pool(name=f"meta_{side['name']}", bufs=1)
    mpool = mctx.__enter__()

    # upfront SBUF loads of idx/seg metadata
    idx_sb = []
    seg_sb = []
    for h in range(2):
        ne = nch_pad[h] * P
        ai = mpool.tile([P, ne // 16], I16, tag=f"ai{h}", name=f"ai{h}")
        si = mpool.tile([P, ne // 16], I16, tag=f"si{h}", name=f"si{h}")
        sg = mpool.tile([P, nch_pad[h]], F32, tag=f"sg{h}", name=f"sg{h}")
        nc.sync.dma_start(out=ai[:], in_=side["t_aidx"][h][:])
        nc.sync.dma_start(out=si[:], in_=side["t_sidx"][h][:])
        nc.sync.dma_start(out=sg[:], in_=side["t_seg"][h][:])
        idx_sb.append((ai, si))
        seg_sb.append(sg)

    # gathers + multiply per group, lo/hi streams interleaved
    prods = [[], []]

    def emit_group(h, g):
        ai, si = idx_sb[h]
        nidx = GROUP * P
        s0 = g * (nidx // 16)
        s1 = (g + 1) * (nidx // 16)
        ga = gpool.tile([P, GROUP * P], F32, tag=f"ga{h}")
        nc.gpsimd.dma_gather(
            out_ap=ga[:].rearrange("p (k d) -> p k d", d=D),
            in_ap=side["a_table"][:],
            idxs_ap=ai[:, s0:s1],
            num_idxs=nidx, num_idxs_reg=nidx, elem_size=D,
            single_packet=False, queue_num=qstate[0] % 4)
        qstate[0] += 1
        gs = gpool.tile([P, GROUP * P], F32, tag=f"gs{h}")
        nc.gpsimd.dma_gather(
            out_ap=gs[:].rearrange("p (k d) -> p k d", d=D),
            in_ap=side["s_table"][h],
            idxs_ap=si[:, s0:s1],
            num_idxs=nidx, num_idxs_reg=nidx, elem_size=D,
            single_packet=False, queue_num=qstate[0] % 4)
        qstate[0] += 1
        nc.vector.tensor_tensor(out=ga[:], in0=ga[:], in1=gs[:],
                                op=mybir.AluOpType.mult)
        prods[h].append(ga)

    ngr = [nch_pad[h] // GROUP for h in range(2)]
    for g in range(max(ngr)):
        for h in range(2):
            if g < ngr[h]:
                emit_group(h, g)

    # batched one-hots: oh_batch[h][b] covers chunks [b*OHB, (b+1)*OHB)
    iota = side["iota"]
    oh_batches = [{}, {}]

    def get_oh(h, b):
        if b not in oh_batches[h]:
            k = min(OHB, nch_pad[h] - b * OHB)
            ohb = wpool.tile([P, OHB * P], F32, tag="ohb", bufs=4)
            seg_b = seg_sb[h][:, b * OHB:b * OHB + k] \
                .rearrange("p (k one) -> p k one", one=1) \
                .to_broadcast([P, k, P])
            iota_b = iota[:].rearrange("p (one d) -> p one d", one=1) \
                .to_broadcast([P, k, P])
            nc.vector.tensor_tensor(
                out=ohb[:, :k * P].rearrange("p (k d) -> p k d", d=P),
                in0=iota_b, in1=seg_b, op=mybir.AluOpType.is_equal)
            oh_batches[h][b] = ohb
        return oh_batches[h][b]

    # per dest-tile: seg matmuls + dense update
    ch_off = [0, 0]
    for t in range(TILES):
        tw = min(P, DPC - t * P)
        psum = ppool.tile([P, P], F32, space="PSUM", tag="pseg")
        nmm = int(cpt[t, 0] + cpt[t, 1])
        mm = 0
        for h in range(2):
            for k in range(int(cpt[t, h])):
                ch = ch_off[h] + k
                g, s = divmod(ch, GROUP)
                b, r = divmod(ch, OHB)
                ohb = get_oh(h, b)
                nc.tensor.matmul(
                    out=psum[:], lhsT=prods[h][g][:, s * P:(s + 1) * P],
                    rhs=ohb[:, r * P:(r + 1) * P],
                    start=(mm == 0), stop=(mm == nmm - 1))
                mm += 1
        ch_off[0] += int(cpt[t, 0])
        ch_off[1] += int(cpt[t, 1])

        segT = wpool.tile([P, P], F32, tag="segT")
        nc.scalar.copy(out=segT[:], in_=psum[:])

        # dense update: out[d, j] = seg.T-term + embed-term
        psum_u = ppool.tile([P, P], F32, space="PSUM", tag="pupd")
        nc.tensor.matmul(out=psum_u[:], lhsT=segT[:], rhs=side["w_seg"][:],
                         start=True, stop=False)
        embT = wpool.tile([P, tw], F32, tag="embT")
        nc.sync.dma_start(out=embT[:], in_=side["embT_sl"][:, t * P:t * P + tw])
        nc.tensor.matmul(out=psum_u[:tw, :], lhsT=embT[:], rhs=side["w_emb"][:],
                         start=False, stop=True)
        out_sb = wpool.tile([P, P], F32, tag="outsb")
        nc.scalar.copy(out=out_sb[:tw, :], in_=psum_u[:tw, :])
        nc.sync.dma_start(out=side["out"][t * P:t * P + tw, :],
                          in_=out_sb[:tw, :])
    mctx.__exit__(None, None, None)


def _build_program(meta):
    """Build the Bass program. meta holds schedules (shared across cores)."""
    nc = bacc.Bacc(num_swdge_queues=4)

    # ---- external inputs ----
    t_a_emb = nc.dram_tensor("a_emb", [N_A, D], F32, kind="ExternalInput")
    t_v_emb = nc.dram_tensor("v_emb", [N_V, D], F32, kind="ExternalInput")
    t_tT = nc.dram_tensor("t_embT", [P, N_T], F32, kind="ExternalInput")
    t_aT = nc.dram_tensor("a_embT", [P, N_A], F32, kind="ExternalInput")
    t_tT_sl = nc.dram_tensor("t_embT_sl", [P, DPC], F32, kind="ExternalInput")
    t_vT_sl = nc.dram_tensor("v_embT_sl", [P, DPC], F32, kind="ExternalInput")
    t_aT_sl = nc.dram_tensor("a_embT_sl", [P, APC], F32, kind="ExternalInput")
    wnames = ["wtT", "watT", "w1aT", "w1bT", "wav", "w2aT", "w2bT", "wa_raw",
              "iota"]
    t_w = {n: nc.dram_tensor(n, [P, P], F32, kind="ExternalInput")
           for n in wnames}

    idx_t = {}
    for sname, sd in (("t", meta["t"]), ("v", meta["v"])):
        for h in range(2):
            ne = sd["nch_pad"][h] * P
            idx_t[(sname, h, "a")] = nc.dram_tensor(
                f"{sname}_aidx{h}", [P, ne // 16], I16, kind="ExternalInput")
            idx_t[(sname, h, "s")] = nc.dram_tensor(
                f"{sname}_sidx{h}", [P, ne // 16], I16, kind="ExternalInput")
            idx_t[(sname, h, "g")] = nc.dram_tensor(
                f"{sname}_seg{h}", [P, sd["nch_pad"][h]], F32,
                kind="ExternalInput")

    # ---- outputs ----
    o_t = nc.dram_tensor("t_upd_part", [DPC, D], F32, kind="ExternalOutput")
    o_v = nc.dram_tensor("v_upd_part", [DPC, D], F32, kind="ExternalOutput")
    o_a = nc.dram_tensor("a_out_part", [APC, D], F32, kind="ExternalOutput")

    # ---- internal DRAM (projections, replicated per core) ----
    d_tproj = nc.dram_tensor("t_proj", [N_T, D], F32)
    d_aproj = nc.dram_tensor("a_proj", [N_A, D], F32)

    with tile.TileContext(nc) as tc:
        with tc.tile_pool(name="consts", bufs=1) as kpool:
            w_sb = {}
            for n in wnames:
                w_sb[n] = kpool.tile([P, P], F32, tag=n, name=n)
                nc.sync.dma_start(out=w_sb[n][:], in_=t_w[n][:])
            # fold w1b_eff.T = (w1[:,128:] @ wa_v).T on device
            with tc.tile_pool(name="fold", bufs=1, space="PSUM") as fpool:
                pf = fpool.tile([P, P], F32, space="PSUM", tag="pf")
                nc.tensor.matmul(out=pf[:], lhsT=w_sb["wav"][:],
                                 rhs=w_sb["w1bT"][:], start=True, stop=True)
                w1beT = kpool.tile([P, P], F32, tag="w1beT")
                nc.scalar.copy(out=w1beT[:], in_=pf[:])

            # ---- phase 1: projections t_proj / a_proj (replicated) ----
            BLK = 2048
            with (
                tc.tile_pool(name="projw", bufs=3) as prw,
                tc.tile_pool(name="projp", bufs=2, space="PSUM") as prp,
            ):
                for (src, dst, n_rows, w_rhs) in (
                        (t_tT, d_tproj, N_T, w_sb["wtT"]),
                        (t_aT, d_aproj, N_A, w_sb["watT"])):
                    nblk = (n_rows + BLK - 1) // BLK
                    for b in range(nblk):
                        r0 = b * BLK
                        bw = min(BLK, n_rows - r0)
                        lhs_big = prw.tile([P, BLK], F32, tag="plhs")
                        nc.sync.dma_start(out=lhs_big[:, :bw],
                                          in_=src[:, r0:r0 + bw])
                        stage = prw.tile([P, BLK], F32, tag="pstg")
                        nsub = (bw + P - 1) // P
                        for qs in range(0, nsub, 4):
                            qe = min(qs + 4, nsub)
                            pp = prp.tile([P, 512], F32, space="PSUM",
                                          tag="ppp")
                            for s in range(qs, qe):
                                sw = min(P, bw - s * P)
                                nc.tensor.matmul(
                                    out=pp[:sw, (s - qs) * P:(s - qs) * P + P],
                                    lhsT=lhs_big[:, s * P:s * P + sw],
                                    rhs=w_rhs[:], start=True, stop=True,
                                    skip_group_check=True)
                            nc.scalar.copy(
                                out=stage[:, qs * P:qs * P + (qe - qs) * P],
                                in_=pp[:, :(qe - qs) * P])
                        if bw % P == 0:
                            nc.sync.dma_start(
                                out=dst[r0:r0 + bw, :].rearrange(
                                    "(s p) d -> p s d", p=P),
                                in_=stage[:, :bw].rearrange(
                                    "p (s d) -> p s d", d=D))
                        else:
                            for s in range(nsub):
                                sw = min(P, bw - s * P)
                                nc.sync.dma_start(
                                    out=dst[r0 + s * P:r0 + s * P + sw, :],
                                    in_=stage[:sw, s * P:(s + 1) * P])

            # ---- phases 2+3: edge aggregation + dense updates ----
            qstate = [0]
            with (
                tc.tile_pool(name="gath", bufs=3) as gpool,
                tc.tile_pool(name="work", bufs=3) as wpool,
                tc.tile_pool(name="psum", bufs=2, space="PSUM") as ppool,
            ):
                # ---- a_out = a_embed @ wa (sharded rows), emitted first ----
                with nc.named_scope("aout"):
                    ntile_a = (APC + P - 1) // P
                    for i in range(ntile_a):
                        r0 = i * P
                        tw = min(P, APC - r0)
                        pa = ppool.tile([P, P], F32, space="PSUM", tag="pa",
                                        name="pa")
                        lhs = wpool.tile([P, tw], F32, tag="albs", name="albs")
                        nc.sync.dma_start(out=lhs[:],
                                          in_=t_aT_sl[:, r0:r0 + tw])
                        nc.tensor.matmul(out=pa[:tw, :], lhsT=lhs[:],
                                         rhs=w_sb["wa_raw"][:], start=True,
                                         stop=True)
                        oa = wpool.tile([P, P], F32, tag="aosb", name="aosb")
                        nc.scalar.copy(out=oa[:tw, :], in_=pa[:tw, :])
                        nc.sync.dma_start(out=o_a[r0:r0 + tw, :],
                                          in_=oa[:tw, :])

                side_t = dict(
                    name="t", cpt=meta["t"]["cpt"],
                    nch_pad=meta["t"]["nch_pad"], qstate=qstate,
                    t_aidx=[idx_t[("t", h, "a")] for h in range(2)],
                    t_sidx=[idx_t[("t", h, "s")] for h in range(2)],
                    t_seg=[idx_t[("t", h, "g")] for h in range(2)],
                    a_table=t_a_emb,
                    s_table=[t_v_emb[0:HALF, :], t_v_emb[HALF:, :]],
                    iota=w_sb["iota"], w_seg=w1beT, w_emb=w_sb["w1aT"],
                    embT_sl=t_tT_sl[:], out=o_t)
                with nc.named_scope("tside"):
                    _emit_side(nc, tc, (gpool, wpool, ppool), side_t)

                side_v = dict(
                    name="v", cpt=meta["v"]["cpt"],
                    nch_pad=meta["v"]["nch_pad"], qstate=qstate,
                    t_aidx=[idx_t[("v", h, "a")] for h in range(2)],
                    t_sidx=[idx_t[("v", h, "s")] for h in range(2)],
                    t_seg=[idx_t[("v", h, "g")] for h in range(2)],
                    a_table=d_aproj,
                    s_table=[d_tproj[0:HALF, :], d_tproj[HALF:, :]],
                    iota=w_sb["iota"], w_seg=w_sb["w2bT"],
                    w_emb=w_sb["w2aT"], embT_sl=t_vT_sl[:], out=o_v)
                with nc.named_scope("vside"):
                    _emit_side(nc, tc, (gpool, wpool, ppool), side_v)


    nc.compile()
    return nc


def _host_prep(inputs):
    """Index-only preprocessing; returns (meta, in_maps)."""
    ptr_t = np.asarray(inputs["ptr_t"])
    ptr_v = np.asarray(inputs["ptr_v"])
    a_l_t = np.asarray(inputs["a_list_t"])
    v_l_t = np.asarray(inputs["v_list_t"])
    a_l_v = np.asarray(inputs["a_list_v"])
    t_l_v = np.asarray(inputs["t_list_v"])
    ar = np.arange(E)
    seg_t = np.searchsorted(ptr_t, ar, side='right') - 1
    seg_v = np.searchsorted(ptr_v, ar, side='right') - 1

    meta = {}
    packed = {}
    for sname, (ptr, al, sl, seg) in (
            ("t", (ptr_t, a_l_t, v_l_t, seg_t)),
            ("v", (ptr_v, a_l_v, t_l_v, seg_v))):
        cpt, nch_pad, streams = _side_schedule(ptr, al, sl, seg)
        meta[sname] = {"cpt": cpt, "nch_pad": nch_pad}
        packed[sname] = streams

    t_embed = np.asarray(inputs["t_embed"], dtype=np.float32)
    v_embed = np.asarray(inputs["v_embed"], dtype=np.float32)
    a_embed = np.asarray(inputs["a_embed"], dtype=np.float32)
    wt = np.asarray(inputs["wt"], dtype=np.float32)
    wa_t = np.asarray(inputs["wa_t"], dtype=np.float32)
    wa_v = np.asarray(inputs["wa_v"], dtype=np.float32)
    w1 = np.asarray(inputs["w1"], dtype=np.float32)
    w2 = np.asarray(inputs["w2"], dtype=np.float32)
    wa = np.asarray(inputs["wa"], dtype=np.float32)

    tT = np.ascontiguousarray(t_embed.T)
    vT = np.ascontiguousarray(v_embed.T)
    aT = np.ascontiguousarray(a_embed.T)
    iota = np.ascontiguousarray(
        np.tile(np.arange(P, dtype=np.float32)[None, :], (P, 1)))

    common = {
        "a_emb": a_embed, "v_emb": v_embed,
        "t_embT": tT, "a_embT": aT,
        "wtT": np.ascontiguousarray(wt.T),
        "watT": np.ascontiguousarray(wa_t.T),
        "w1aT": np.ascontiguousarray(w1[:, :D].T),
        "w1bT": np.ascontiguousarray(w1[:, D:].T),
        "wav": wa_v,
        "w2aT": np.ascontiguousarray(w2[:, :D].T),
        "w2bT": np.ascontiguousarray(w2[:, D:].T),
        "wa_raw": wa,
        "iota": iota,
    }

    in_maps = []
    for c in range(NC):
        m = dict(common)
        m["t_embT_sl"] = np.ascontiguousarray(tT[:, c * DPC:(c + 1) * DPC])
        m["v_embT_sl"] = np.ascontiguousarray(vT[:, c * DPC:(c + 1) * DPC])
        m["a_embT_sl"] = np.ascontiguousarray(aT[:, c * APC:(c + 1) * APC])
        for sname in ("t", "v"):
            for h in range(2):
                a_idx, s_idx, segl = packed[sname][c][h]
                m[f"{sname}_aidx{h}"] = _pack_idx(a_idx)
                m[f"{sname}_sidx{h}"] = _pack_idx(s_idx)
                m[f"{sname}_seg{h}"] = _pack_seg(segl)
        in_maps.append(m)
    return meta, in_maps


_CACHE = {}


def _get_compiled(inputs):
    key = (inputs["ptr_t"].tobytes()[:4096], inputs["ptr_v"].tobytes()[:4096],
           inputs["a_list_t"].tobytes()[:4096])
    import hashlib
    key = hashlib.sha1(b"".join(key)).hexdigest()
    if key not in _CACHE:
        meta, in_maps = _host_prep(inputs)
        nc = _build_program(meta)
        _CACHE[key] = (nc, meta)
    else:
        nc, meta = _CACHE[key]
        _, in_maps = _host_prep(inputs)
    return _CACHE[key][0], in_maps


def run(inputs, trace=False):
    nc, in_maps = _get_compiled(inputs)
    res = run_bass_kernel_spmd(nc, in_maps, list(range(NC)), trace=trace)
    t_upd = np.concatenate([res.results[c]["t_upd_part"] for c in range(NC)])
    v_upd = np.concatenate([res.results[c]["v_upd_part"] for c in range(NC)])
    a_out = np.concatenate([res.results[c]["a_out_part"] for c in range(NC)])
    return (t_upd, v_upd, a_out), res


def kernel(**inputs):
    out, _ = run(inputs, trace=False)
    return out


# revision 10
# speedup vs baseline: 1.8496x; 1.2323x over previous
"""Trainium2 Bass kernel for nn_Aggregator1 (GNN message passing).

Strategy (8 NeuronCores, SPMD, no collectives):
  - Destination nodes are split evenly across cores (6250 t-rows, 6250 v-rows,
    2500 a-rows per core). Each core processes exactly the edges that land in
    its destination slice, so no cross-core reduction is needed.
  - Edge gathers use the custom dma_gather instruction (int16 indices,
    4 SWDGE queues round-robin). 50000-row tables are addressed as lo/hi
    halves (25000 rows each) so indices fit int16.
  - Per 128-edge chunk: DVE elementwise multiply, DVE one-hot (is_equal vs
    iota), PE matmul products.T @ onehot accumulating seg.T per 128-dest tile
    in PSUM (transposed output avoids any activation transposes).
  - v-side projections (t_embed @ wt.T, a_embed @ wa_t.T) are computed
    replicated on every core into internal DRAM, then gathered per edge.
  - Dense updates use host-transposed embedding tables as matmul lhsT.
Host-side prep is index manipulation only (no float math on edge data).
"""
import sys
import types

import numpy as np

# ---- shim: provide antenv.axon_hooks (absent in this image) ----
if 'antenv.axon_hooks' not in sys.modules:
    _m = types.ModuleType('antenv.axon_hooks')
    _m._hook = None
    _m.set_axon_ntff_profile_hook = lambda h: setattr(_m, '_hook', h)
    _m.get_axon_ntff_profile_hook = lambda: _m._hook
    sys.modules['antenv.axon_hooks'] = _m
    try:
        from trn_agent_boot.trn_boot import _ntff_profile_via_ctypes
        _m.set_axon_ntff_profile_hook(
            _ntff_profile_via_ctypes('/opt/axon/libaxon_pjrt.so'))
    except Exception:
        pass

import concourse.bass as bass
import concourse.bacc as bacc
import concourse.mybir as mybir
import concourse.tile as tile
import concourse.bass_utils as bass_utils
from concourse.bass_utils import run_bass_kernel_spmd

bass_utils.upload_artifacts = lambda tmpdir: "local://" + str(tmpdir)

P = 128
D = 128
N_T = 50000
N_V = 50000
N_A = 20000
E = 640000
NC = 8
DPC = N_T // NC            # dest nodes per core (t and v sides)
APC = N_A // NC            # a rows per core
TILES = (DPC + P - 1) // P  # dest tiles per core per side (49; last has 106)
HALF = 25000               # lo/hi split for 50000-row gather tables
GROUP = 16                 # chunks per gather group (16*128 = 2048 edges)
F32 = mybir.dt.float32
I16 = mybir.dt.int16


def _side_schedule(ptr, a_list, s_list, seg):
    """Build the per-core chunk schedule for one aggregation side.

    Returns (cpt, streams) where cpt[t][h] is the static chunk count for
    dest-tile t, source-half h, and streams[c][h] are per-core flat arrays
    (a_idx int16, s_idx int16, seg_local float32) laid out chunk-major.
    """
    # per (core, tile, half) edge index arrays
    per = [[[None, None] for _ in range(TILES)] for _ in range(NC)]
    for c in range(NC):
        base_d = c * DPC
        for t in range(TILES):
            d0 = base_d + t * P
            d1 = min(base_d + (t + 1) * P, base_d + DPC)
            e0, e1 = int(ptr[d0]), int(ptr[d1])
            sl = s_list[e0:e1]
            al = a_list[e0:e1]
            dl = seg[e0:e1] - d0  # local dest in [0, d1-d0)
            lo = sl < HALF
            per[c][t][0] = (al[lo], sl[lo], dl[lo])
            per[c][t][1] = (al[~lo], sl[~lo] - HALF, dl[~lo])

    cpt = np.zeros((TILES, 2), dtype=np.int64)
    for t in range(TILES):
        for h in range(2):
            mx = max(len(per[c][t][h][0]) for c in range(NC))
            cpt[t, h] = (mx + P - 1) // P
    # ensure every tile has >= 1 chunk so its PSUM gets start=True zeroing
    for t in range(TILES):
        if cpt[t, 0] + cpt[t, 1] == 0:
            cpt[t, 0] = 1

    nch = [int(cpt[:, h].sum()) for h in range(2)]
    nch_pad = [((n + GROUP - 1) // GROUP) * GROUP for n in nch]

    streams = []
    for c in range(NC):
        halves = []
        for h in range(2):
            ne = nch_pad[h] * P
            a_idx = np.zeros(ne, dtype=np.int16)
            s_idx = np.zeros(ne, dtype=np.int16)
            segl = np.full(ne, -1.0, dtype=np.float32)
            off = 0
            for t in range(TILES):
                al, sl, dl = per[c][t][h]
                n = len(al)
                a_idx[off:off + n] = al
                s_idx[off:off + n] = sl
                segl[off:off + n] = dl
                off += int(cpt[t, h]) * P
            halves.append((a_idx, s_idx, segl))
        streams.append(halves)
    return cpt, nch_pad, streams


def _pack_idx(idx_flat):
    """int16 flat [NE] -> [128, NE//16] tile layout for dma_gather."""
    ne = idx_flat.shape[0]
    arr = idx_flat.reshape(ne // 16, 16).T  # [16, S]
    return np.ascontiguousarray(np.tile(arr, (8, 1)))  # [128, S]


def _pack_seg(seg_flat):
    """f32 flat [NE] -> [128, NCH] (chunk ch, partition p) = seg[ch*128+p]."""
    ne = seg_flat.shape[0]
    return np.ascontiguousarray(seg_flat.reshape(ne // P, P).T)


OHB = 16  # chunks per batched one-hot op


def _emit_side(nc, tc, pools, side):
    """Emit gathers + multiply + onehot + seg matmuls + dense updates."""
    (gpool, wpool, ppool) = pools
    cpt = side["cpt"]
    nch_pad = side["nch_pad"]
    qstate = side["qstate"]
    mctx = tc.tile_pool(name=f"meta_{side['name']}", bufs=1)
    mpool = mctx.__enter__()

    # upfront SBUF loads of idx/seg metadata
    idx_sb = []
    seg_sb = []
    for h in range(2):
        ne = nch_pad[h] * P
        ai = mpool.tile([P, ne // 16], I16, tag=f"ai{h}", name=f"ai{h}")
        si = mpool.tile([P, ne // 16], I16, tag=f"si{h}", name=f"si{h}")
        sg = mpool.tile([P, nch_pad[h]], F32, tag=f"sg{h}", name=f"sg{h}")
        nc.scalar.dma_start(out=ai[:], in_=side["t_aidx"][h][:])
        nc.scalar.dma_start(out=si[:], in_=side["t_sidx"][h][:])
        nc.scalar.dma_start(out=sg[:], in_=side["t_seg"][h][:])
        idx_sb.append((ai, si))
        seg_sb.append(sg)

    # gathers + multiply per group, lo/hi streams interleaved
    prods = [[], []]

    def emit_group(h, g):
        ai, si = idx_sb[h]
        nidx = GROUP * P
        s0 = g * (nidx // 16)
        s1 = (g + 1) * (nidx // 16)
        ga = gpool.tile([P, GROUP * P], F32, tag=f"ga{h}")
        nc.gpsimd.dma_gather(
            out_ap=ga[:].rearrange("p (k d) -> p k d", d=D),
            in_ap=side["a_table"][:],
            idxs_ap=ai[:, s0:s1],
            num_idxs=nidx, num_idxs_reg=nidx, elem_size=D,
            single_packet=False, queue_num=qstate[0] % 4)
        qstate[0] += 1
        gs = gpool.tile([P, GROUP * P], F32, tag=f"gs{h}")
        nc.gpsimd.dma_gather(
            out_ap=gs[:].rearrange("p (k d) -> p k d", d=D),
            in_ap=side["s_table"][h],
            idxs_ap=si[:, s0:s1],
            num_idxs=nidx, num_idxs_reg=nidx, elem_size=D,
            single_packet=False, queue_num=qstate[0] % 4)
        qstate[0] += 1
        nc.vector.tensor_tensor(out=ga[:], in0=ga[:], in1=gs[:],
                                op=mybir.AluOpType.mult)
        prods[h].append(ga)

    ngr = [nch_pad[h] // GROUP for h in range(2)]
    for g in range(max(ngr)):
        for h in range(2):
            if g < ngr[h]:
                emit_group(h, g)

    # batched one-hots: oh_batch[h][b] covers chunks [b*OHB, (b+1)*OHB)
    iota = side["iota"]
    oh_batches = [{}, {}]

    def get_oh(h, b):
        if b not in oh_batches[h]:
            k = min(OHB, nch_pad[h] - b * OHB)
            ohb = wpool.tile([P, OHB * P], F32, tag="ohb", bufs=4)
            seg_b = seg_sb[h][:, b * OHB:b * OHB + k] \
                .rearrange("p (k one) -> p k one", one=1) \
                .to_broadcast([P, k, P])
            iota_b = iota[:].rearrange("p (one d) -> p one d", one=1) \
                .to_broadcast([P, k, P])
            nc.vector.tensor_tensor(
                out=ohb[:, :k * P].rearrange("p (k d) -> p k d", d=P),
                in0=iota_b, in1=seg_b, op=mybir.AluOpType.is_equal)
            oh_batches[h][b] = ohb
        return oh_batches[h][b]

    # per dest-tile: seg matmuls + dense update
    ch_off = [0, 0]
    for t in range(TILES):
        tw = min(P, DPC - t * P)
        psum = ppool.tile([P, P], F32, space="PSUM", tag="pseg")
        nmm = int(cpt[t, 0] + cpt[t, 1])
        mm = 0
        for h in range(2):
            for k in range(int(cpt[t, h])):
                ch = ch_off[h] + k
                g, s = divmod(ch, GROUP)
                b, r = divmod(ch, OHB)
                ohb = get_oh(h, b)
                nc.tensor.matmul(
                    out=psum[:], lhsT=prods[h][g][:, s * P:(s + 1) * P],
                    rhs=ohb[:, r * P:(r + 1) * P],
                    start=(mm == 0), stop=(mm == nmm - 1))
                mm += 1
        ch_off[0] += int(cpt[t, 0])
        ch_off[1] += int(cpt[t, 1])

        segT = wpool.tile([P, P], F32, tag="segT")
        nc.scalar.copy(out=segT[:], in_=psum[:])

        # dense update: out[d, j] = seg.T-term + embed-term
        psum_u = ppool.tile([P, P], F32, space="PSUM", tag="pupd")
        nc.tensor.matmul(out=psum_u[:], lhsT=segT[:], rhs=side["w_seg"][:],
                         start=True, stop=False)
        embT = wpool.tile([P, tw], F32, tag="embT")
        nc.sync.dma_start(out=embT[:], in_=side["embT_sl"][:, t * P:t * P + tw])
        nc.tensor.matmul(out=psum_u[:tw, :], lhsT=embT[:], rhs=side["w_emb"][:],
                         start=False, stop=True)
        out_sb = wpool.tile([P, P], F32, tag="outsb")
        nc.scalar.copy(out=out_sb[:tw, :], in_=psum_u[:tw, :])
        nc.sync.dma_start(out=side["out"][t * P:t * P + tw, :],
                          in_=out_sb[:tw, :])
    mctx.__exit__(None, None, None)


def _build_program(meta):
    """Build the Bass program. meta holds schedules (shared across cores)."""
    nc = bacc.Bacc(num_swdge_queues=4)

    # ---- external inputs ----
    t_a_emb = nc.dram_tensor("a_emb", [N_A, D], F32, kind="ExternalInput")
    t_v_emb = nc.dram_tensor("v_emb", [N_V, D], F32, kind="ExternalInput")
    t_tT = nc.dram_tensor("t_embT", [P, N_T], F32, kind="ExternalInput")
    t_aT = nc.dram_tensor("a_embT", [P, N_A], F32, kind="ExternalInput")
    t_tT_sl = nc.dram_tensor("t_embT_sl", [P, DPC], F32, kind="ExternalInput")
    t_vT_sl = nc.dram_tensor("v_embT_sl", [P, DPC], F32, kind="ExternalInput")
    t_aT_sl = nc.dram_tensor("a_embT_sl", [P, APC], F32, kind="ExternalInput")
    wnames = ["wtT", "watT", "w1aT", "w1bT", "wav", "w2aT", "w2bT", "wa_raw",
              "iota"]
    t_w = {n: nc.dram_tensor(n, [P, P], F32, kind="ExternalInput")
           for n in wnames}

    idx_t = {}
    for sname, sd in (("t", meta["t"]), ("v", meta["v"])):
        for h in range(2):
            ne = sd["nch_pad"][h] * P
            idx_t[(sname, h, "a")] = nc.dram_tensor(
                f"{sname}_aidx{h}", [P, ne // 16], I16, kind="ExternalInput")
            idx_t[(sname, h, "s")] = nc.dram_tensor(
                f"{sname}_sidx{h}", [P, ne // 16], I16, kind="ExternalInput")
            idx_t[(sname, h, "g")] = nc.dram_tensor(
                f"{sname}_seg{h}", [P, sd["nch_pad"][h]], F32,
                kind="ExternalInput")

    # ---- outputs ----
    o_t = nc.dram_tensor("t_upd_part", [DPC, D], F32, kind="ExternalOutput")
    o_v = nc.dram_tensor("v_upd_part", [DPC, D], F32, kind="ExternalOutput")
    o_a = nc.dram_tensor("a_out_part", [APC, D], F32, kind="ExternalOutput")

    # ---- internal DRAM (projections, replicated per core) ----
    d_tproj = nc.dram_tensor("t_proj", [N_T, D], F32)
    d_aproj = nc.dram_tensor("a_proj", [N_A, D], F32)

    with tile.TileContext(nc) as tc:
        with tc.tile_pool(name="consts", bufs=1) as kpool:
            w_sb = {}
            for n in wnames:
                w_sb[n] = kpool.tile([P, P], F32, tag=n, name=n)
                nc.sync.dma_start(out=w_sb[n][:], in_=t_w[n][:])
            # fold w1b_eff.T = (w1[:,128:] @ wa_v).T on device
            with tc.tile_pool(name="fold", bufs=1, space="PSUM") as fpool:
                pf = fpool.tile([P, P], F32, space="PSUM", tag="pf")
                nc.tensor.matmul(out=pf[:], lhsT=w_sb["wav"][:],
                                 rhs=w_sb["w1bT"][:], start=True, stop=True)
                w1beT = kpool.tile([P, P], F32, tag="w1beT")
                nc.scalar.copy(out=w1beT[:], in_=pf[:])

            # ---- phase 1: projections t_proj / a_proj (replicated) ----
            BLK = 2048
            with (
                tc.tile_pool(name="projw", bufs=3) as prw,
                tc.tile_pool(name="projp", bufs=2, space="PSUM") as prp,
            ):
                for (src, dst, n_rows, w_rhs) in (
                        (t_tT, d_tproj, N_T, w_sb["wtT"]),
                        (t_aT, d_aproj, N_A, w_sb["watT"])):
                    nblk = (n_rows + BLK - 1) // BLK
                    for b in range(nblk):
                        r0 = b * BLK
                        bw = min(BLK, n_rows - r0)
                        lhs_big = prw.tile([P, BLK], F32, tag="plhs")
                        nc.sync.dma_start(out=lhs_big[:, :bw],
                                          in_=src[:, r0:r0 + bw])
                        stage = prw.tile([P, BLK], F32, tag="pstg")
                        nsub = (bw + P - 1) // P
                        for qs in range(0, nsub, 4):
                            qe = min(qs + 4, nsub)
                            pp = prp.tile([P, 512], F32, space="PSUM",
                                          tag="ppp")
                            for s in range(qs, qe):
                                sw = min(P, bw - s * P)
                                nc.tensor.matmul(
                                    out=pp[:sw, (s - qs) * P:(s - qs) * P + P],
                                    lhsT=lhs_big[:, s * P:s * P + sw],
                                    rhs=w_rhs[:], start=True, stop=True,
                                    skip_group_check=True)
                            nc.scalar.copy(
                                out=stage[:, qs * P:qs * P + (qe - qs) * P],
                                in_=pp[:, :(qe - qs) * P])
                        if bw % P == 0:
                            nc.sync.dma_start(
                                out=dst[r0:r0 + bw, :].rearrange(
                                    "(s p) d -> p s d", p=P),
                                in_=stage[:, :bw].rearrange(
                                    "p (s d) -> p s d", d=D))
                        else:
                            for s in range(nsub):
                                sw = min(P, bw - s * P)
                                nc.sync.dma_start(
                                    out=dst[r0 + s * P:r0 + s * P + sw, :],
                                    in_=stage[:sw, s * P:(s + 1) * P])

            # ---- phases 2+3: edge aggregation + dense updates ----
            qstate = [0]
            with (
                tc.tile_pool(name="gath", bufs=4) as gpool,
                tc.tile_pool(name="work", bufs=3) as wpool,
                tc.tile_pool(name="psum", bufs=2, space="PSUM") as ppool,
            ):
                # ---- a_out = a_embed @ wa (sharded rows), emitted first ----
                with nc.named_scope("aout"):
                    ntile_a = (APC + P - 1) // P
                    for i in range(ntile_a):
                        r0 = i * P
                        tw = min(P, APC - r0)
                        pa = ppool.tile([P, P], F32, space="PSUM", tag="pa",
                                        name="pa")
                        lhs = wpool.tile([P, tw], F32, tag="albs", name="albs")
                        nc.sync.dma_start(out=lhs[:],
                                          in_=t_aT_sl[:, r0:r0 + tw])
                        nc.tensor.matmul(out=pa[:tw, :], lhsT=lhs[:],
                                         rhs=w_sb["wa_raw"][:], start=True,
                                         stop=True)
                        oa = wpool.tile([P, P], F32, tag="aosb", name="aosb")
                        nc.scalar.copy(out=oa[:tw, :], in_=pa[:tw, :])
                        nc.sync.dma_start(out=o_a[r0:r0 + tw, :],
                                          in_=oa[:tw, :])

                side_t = dict(
                    name="t", cpt=meta["t"]["cpt"],
                    nch_pad=meta["t"]["nch_pad"], qstate=qstate,
                    t_aidx=[idx_t[("t", h, "a")] for h in range(2)],
                    t_sidx=[idx_t[("t", h, "s")] for h in range(2)],
                    t_seg=[idx_t[("t", h, "g")] for h in range(2)],
                    a_table=t_a_emb,
                    s_table=[t_v_emb[0:HALF, :], t_v_emb[HALF:, :]],
                    iota=w_sb["iota"], w_seg=w1beT, w_emb=w_sb["w1aT"],
                    embT_sl=t_tT_sl[:], out=o_t)
                with nc.named_scope("tside"):
                    _emit_side(nc, tc, (gpool, wpool, ppool), side_t)

                side_v = dict(
                    name="v", cpt=meta["v"]["cpt"],
                    nch_pad=meta["v"]["nch_pad"], qstate=qstate,
                    t_aidx=[idx_t[("v", h, "a")] for h in range(2)],
                    t_sidx=[idx_t[("v", h, "s")] for h in range(2)],
                    t_seg=[idx_t[("v", h, "g")] for h in range(2)],
                    a_table=d_aproj,
                    s_table=[d_tproj[0:HALF, :], d_tproj[HALF:, :]],
                    iota=w_sb["iota"], w_seg=w_sb["w2bT"],
                    w_emb=w_sb["w2aT"], embT_sl=t_vT_sl[:], out=o_v)
                with nc.named_scope("vside"):
                    _emit_side(nc, tc, (gpool, wpool, ppool), side_v)


    nc.compile()
    return nc


def _host_prep(inputs):
    """Index-only preprocessing; returns (meta, in_maps)."""
    ptr_t = np.asarray(inputs["ptr_t"])
    ptr_v = np.asarray(inputs["ptr_v"])
    a_l_t = np.asarray(inputs["a_list_t"])
    v_l_t = np.asarray(inputs["v_list_t"])
    a_l_v = np.asarray(inputs["a_list_v"])
    t_l_v = np.asarray(inputs["t_list_v"])
    ar = np.arange(E)
    seg_t = np.searchsorted(ptr_t, ar, side='right') - 1
    seg_v = np.searchsorted(ptr_v, ar, side='right') - 1

    meta = {}
    packed = {}
    for sname, (ptr, al, sl, seg) in (
            ("t", (ptr_t, a_l_t, v_l_t, seg_t)),
            ("v", (ptr_v, a_l_v, t_l_v, seg_v))):
        cpt, nch_pad, streams = _side_schedule(ptr, al, sl, seg)
        meta[sname] = {"cpt": cpt, "nch_pad": nch_pad}
        packed[sname] = streams

    t_embed = np.asarray(inputs["t_embed"], dtype=np.float32)
    v_embed = np.asarray(inputs["v_embed"], dtype=np.float32)
    a_embed = np.asarray(inputs["a_embed"], dtype=np.float32)
    wt = np.asarray(inputs["wt"], dtype=np.float32)
    wa_t = np.asarray(inputs["wa_t"], dtype=np.float32)
    wa_v = np.asarray(inputs["wa_v"], dtype=np.float32)
    w1 = np.asarray(inputs["w1"], dtype=np.float32)
    w2 = np.asarray(inputs["w2"], dtype=np.float32)
    wa = np.asarray(inputs["wa"], dtype=np.float32)

    tT = np.ascontiguousarray(t_embed.T)
    vT = np.ascontiguousarray(v_embed.T)
    aT = np.ascontiguousarray(a_embed.T)
    iota = np.ascontiguousarray(
        np.tile(np.arange(P, dtype=np.float32)[None, :], (P, 1)))

    common = {
        "a_emb": a_embed, "v_emb": v_embed,
        "t_embT": tT, "a_embT": aT,
        "wtT": np.ascontiguousarray(wt.T),
        "watT": np.ascontiguousarray(wa_t.T),
        "w1aT": np.ascontiguousarray(w1[:, :D].T),
        "w1bT": np.ascontiguousarray(w1[:, D:].T),
        "wav": wa_v,
        "w2aT": np.ascontiguousarray(w2[:, :D].T),
        "w2bT": np.ascontiguousarray(w2[:, D:].T),
        "wa_raw": wa,
        "iota": iota,
    }

    in_maps = []
    for c in range(NC):
        m = dict(common)
        m["t_embT_sl"] = np.ascontiguousarray(tT[:, c * DPC:(c + 1) * DPC])
        m["v_embT_sl"] = np.ascontiguousarray(vT[:, c * DPC:(c + 1) * DPC])
        m["a_embT_sl"] = np.ascontiguousarray(aT[:, c * APC:(c + 1) * APC])
        for sname in ("t", "v"):
            for h in range(2):
                a_idx, s_idx, segl = packed[sname][c][h]
                m[f"{sname}_aidx{h}"] = _pack_idx(a_idx)
                m[f"{sname}_sidx{h}"] = _pack_idx(s_idx)
                m[f"{sname}_seg{h}"] = _pack_seg(segl)
        in_maps.append(m)
    return meta, in_maps


_CACHE = {}


def _get_compiled(inputs):
    key = (inputs["ptr_t"].tobytes()[:4096], inputs["ptr_v"].tobytes()[:4096],
           inputs["a_list_t"].tobytes()[:4096])
    import hashlib
    key = hashlib.sha1(b"".join(key)).hexdigest()
    if key not in _CACHE:
        meta, in_maps = _host_prep(inputs)
        nc = _build_program(meta)
        _CACHE[key] = (nc, meta)
    else:
        nc, meta = _CACHE[key]
        _, in_maps = _host_prep(inputs)
    return _CACHE[key][0], in_maps


def run(inputs, trace=False):
    nc, in_maps = _get_compiled(inputs)
    res = run_bass_kernel_spmd(nc, in_maps, list(range(NC)), trace=trace)
    t_upd = np.concatenate([res.results[c]["t_upd_part"] for c in range(NC)])
    v_upd = np.concatenate([res.results[c]["v_upd_part"] for c in range(NC)])
    a_out = np.concatenate([res.results[c]["a_out_part"] for c in range(NC)])
    return (t_upd, v_upd, a_out), res


def kernel(**inputs):
    out, _ = run(inputs, trace=False)
    return out
